# revision 31
# baseline (speedup 1.0000x reference)
"""Trainium2 Bass kernel for DiffMultiHeadedAttention (differential attention).

Model (per reference):
    q = x @ Wq.T + bq; k = ef @ Wk.T + bk; v = ef @ Wv.T + bv
    lambda_full = exp(sum(lq1*lk1)) - exp(sum(lq2*lk2)) + 0.8
    att  = softmax(causal_mask(q_hh @ k_hh.T / sqrt(32)))   per 32 half-heads
    out_h = (att[2h] - lambda_full * att[2h+1]) @ v_h       per 16 heads
B=4, T=N=1024, H=16 heads of 64, 2H=32 half-heads of 32.

Sharding over 8 cores: core c = (batch b = c//2, head-group hg = c%2).
Each core owns one batch element and 8 full heads (16 half-heads) and
computes out^T [512, 1024] (fp16); the host casts/transposes/reassembles.

v3 design notes (HW-measured):
  - GpSimd per-instruction dispatch is ~2.8us -> no per-slot gpsimd ops at
    all.  The softmax denominator is replicated onto PSUM partitions
    64:128 by 64 ones-columns in the AV stationary (matmul-side
    broadcast); a partition-misaligned DVE copy (measured legal: out/in
    partition bases may differ, and PSUM+SBUF operands may be misaligned;
    only SBUF+SBUF input pairs must align) brings it to partitions 0:64
    for an aligned reciprocal, then the combine multiplies straight out
    of PSUM with the -lambda fold fused via scalar_tensor_tensor.
  - Slots are (head, t-half); QK runs 2-way row-tiled with one
    [128,2,512] PSUM claim per n-tile so the claim rotation gives one
    n-tile of lookahead and the exp stream stays back-to-back.
  - Projection chains and AV sweeps are filler units interleaved between
    QK groups, hand-scheduled so chain results land just before the slot
    that needs them (deadlines audited below).
  - Input DMAs: priority waves (wk,ef on sync / wq,x on scalar / wv on
    gpsimd triggers) so the first exp fires as early as possible.
  - PSUM: qk [128,2,512]x2 (4 banks) + av [128,512]x2 (2) + pj
    [128,512]x2 (2) = 8 banks.
"""

import math

import numpy as np

B, T, N, HIDDEN = 4, 1024, 1024, 1024
H, HEAD, HALF = 16, 64, 32
O = 512            # per-core hidden slice (8 heads * 64)
HPC = 8            # heads per core
LAMBDA_INIT = 0.8
SCALE = 1.0 / math.sqrt(HALF)
P = 128
IC = HIDDEN // P   # 8 contraction chunks
OC = O // P        # 4 output chunks of the projections
NT = N // P        # 8 n-tiles (keys)
NCORES = 8

_STATE = {}


def _build_nc():
    from contextlib import ExitStack

    import concourse.bacc as bacc
    import concourse.mybir as mybir
    import concourse.tile as tile
    from concourse.bass import ts

    f32 = mybir.dt.float32
    f16 = mybir.dt.float16
    AF = mybir.ActivationFunctionType
    ALU = mybir.AluOpType

    nc = bacc.Bacc("TRN2", target_bir_lowering=False, debug=False)

    xt_d = nc.dram_tensor("xt", [HIDDEN, T], f16, kind="ExternalInput")
    eft_d = nc.dram_tensor("eft", [HIDDEN, N], f16, kind="ExternalInput")
    wqt_d = nc.dram_tensor("wqt", [HIDDEN, O], f16, kind="ExternalInput")
    wkt_d = nc.dram_tensor("wkt", [HIDDEN, O], f16, kind="ExternalInput")
    wvt_d = nc.dram_tensor("wvt", [HIDDEN, O], f16, kind="ExternalInput")
    bq_d = nc.dram_tensor("bq", [1, O], f32, kind="ExternalInput")
    bk_d = nc.dram_tensor("bk", [1, O], f32, kind="ExternalInput")
    bv_d = nc.dram_tensor("bv", [1, O], f32, kind="ExternalInput")
    lq1_d = nc.dram_tensor("lq1", [1, HALF], f32, kind="ExternalInput")
    lq2_d = nc.dram_tensor("lq2", [1, HALF], f32, kind="ExternalInput")
    lk1_d = nc.dram_tensor("lk1", [1, HALF], f32, kind="ExternalInput")
    lk2_d = nc.dram_tensor("lk2", [1, HALF], f32, kind="ExternalInput")
    outT_d = nc.dram_tensor("outT", [O, T], f16, kind="ExternalOutput")

    with tile.TileContext(nc) as tc:
        with ExitStack() as ctx:
            const = ctx.enter_context(tc.tile_pool(name="const", bufs=1))

            # ---- input loads: per-ic DMAs in priority waves, triggers spread
            # over three DMA-capable sequencers ----
            big = ctx.enter_context(tc.tile_pool(name="big", bufs=1))
            efT = big.tile([P, IC, N], f16)
            wkT = big.tile([P, IC, O], f16)
            xT = big.tile([P, IC, T], f16)
            wqT = big.tile([P, IC, O], f16)
            wvT = big.tile([P, IC, O], f16)

            # tiny lambda/bias transfers FIRST: the lambda exp heads the
            # scalar-engine FIFO and the bias tiles gate the first chains'
            # bias-adds, so these must not queue behind the big waves
            lam_in = const.tile([1, 4, HALF], f32)
            nc.sync.dma_start(lam_in[:, 0, :], lq1_d[:])
            nc.sync.dma_start(lam_in[:, 1, :], lk1_d[:])
            nc.sync.dma_start(lam_in[:, 2, :], lq2_d[:])
            nc.sync.dma_start(lam_in[:, 3, :], lk2_d[:])
            bq_sb = const.tile([P, OC], f32)
            nc.gpsimd.dma_start(bq_sb, bq_d[0].rearrange("(a p) -> p a", p=P))
            bk_sb = const.tile([P, OC], f32)
            nc.sync.dma_start(bk_sb, bk_d[0].rearrange("(a p) -> p a", p=P))
            bv_1 = const.tile([1, O], f32)
            nc.gpsimd.dma_start(bv_1, bv_d[:])

            # paired-ic transfers minimize trigger count (~0.7us serial per
            # dma_start on the sequencer); k-path (wk, ef) first, q-path
            # (wq, x) on the scalar sequencer in parallel, wv last.
            def load_pair(eng, dst, src_d, icp):
                eng.dma_start(
                    dst[:, 2 * icp : 2 * icp + 2, :],
                    src_d[ts(icp, 2 * P), :].rearrange("(a p) n -> p a n", p=P),
                )

            def load_pair_cols(eng, dst, src_d, icp, c0):
                eng.dma_start(
                    dst[:, 2 * icp : 2 * icp + 2, c0 : c0 + 512],
                    src_d[ts(icp, 2 * P), c0 : c0 + 512].rearrange(
                        "(a p) n -> p a n", p=P
                    ),
                )

            # k0(0)/q0(0) need weights + activation cols 0:512 only.  The
            # scalar sequencer carries NO dma triggers (a trigger is ~0.8us
            # and would stall the exp stream behind it); q-path goes via
            # gpsimd's sequencer instead.
            for icp in range(IC // 2):
                load_pair(nc.sync, wkT, wkt_d, icp)
                load_pair_cols(nc.sync, efT, eft_d, icp, 0)
                load_pair(nc.gpsimd, wqT, wqt_d, icp)
                load_pair_cols(nc.gpsimd, xT, xt_d, icp, 0)
            for icp in range(IC // 2):
                load_pair_cols(nc.sync, efT, eft_d, icp, 512)
                load_pair_cols(nc.gpsimd, xT, xt_d, icp, 512)
            for icp in range(IC // 2):
                load_pair(nc.sync, wvT, wvt_d, icp)

            # warm-up operand first in the DVE FIFO so the PE warm-up
            # matmuls can start before the lambda DMA lands
            warm1 = const.tile([1, 64], f16)
            nc.vector.memset(warm1, 0.0)

            # ---- lambda_full (tiny, computed once) ----
            lam_tmp = const.tile([1, 2, HALF], f32)
            nc.vector.tensor_mul(lam_tmp[:, 0, :], lam_in[:, 0, :], lam_in[:, 1, :])
            nc.vector.tensor_mul(lam_tmp[:, 1, :], lam_in[:, 2, :], lam_in[:, 3, :])
            lam_s = const.tile([1, 2], f32)
            nc.vector.tensor_reduce(
                lam_s, lam_tmp, axis=mybir.AxisListType.X, op=ALU.add
            )
            lam_e = const.tile([1, 2], f32)
            nc.scalar.activation(lam_e, lam_s, AF.Exp)
            # lam_neg = -(e1 - e2 + 0.8) = e2 - e1 - 0.8
            lam_neg = const.tile([1, 1], f32)
            nc.vector.tensor_sub(lam_neg, lam_e[:, 1:2], lam_e[:, 0:1])
            nc.vector.tensor_scalar_add(lam_neg, lam_neg, -LAMBDA_INIT)
            lam_neg64 = const.tile([64, 1], f32)
            nc.gpsimd.partition_broadcast(lam_neg64, lam_neg)

            # 0/1 upper-triangular mask (keep t_local >= n_local), doubled
            # along a middle dim so one DVE mul masks both half-heads.
            tri2 = const.tile([P, 2, P], f16)
            neg3 = const.tile([P, 1], f32)
            nc.vector.memset(neg3, -3.0)
            nc.gpsimd.memset(tri2, 1.0)
            nc.gpsimd.affine_select(
                out=tri2,
                in_=tri2,
                compare_op=ALU.is_ge,
                fill=0.0,
                base=0,
                pattern=[[0, 2], [1, P]],
                channel_multiplier=-1,
            )

            # ---- v bias broadcast ----
            bvb = const.tile([P, O], f32)
            nc.gpsimd.partition_broadcast(bvb, bv_1)

            # ---- persistent projection outputs ----
            proj = ctx.enter_context(tc.tile_pool(name="proj", bufs=1))
            qT = proj.tile([P, OC, T], f16)          # [d-part, oc, t]
            kT = proj.tile([P, OC, N], f16)          # [d-part, oc, n]
            # [n-part, nt, h, v(64) | ones(64)]: the ones columns make the AV
            # matmul emit the softmax denominator replicated on partitions
            # 64:128 (matmul-side partition broadcast).
            vaug = proj.tile([P, NT, HPC, 2 * HEAD], f16)
            nc.vector.memset(vaug[:, :, :, HEAD : 2 * HEAD], 1.0)

            # ---- PSUM pools (8 banks total) ----
            ps_qk = ctx.enter_context(
                tc.tile_pool(name="ps_qk", bufs=2, space="PSUM")
            )
            ps_av = ctx.enter_context(
                tc.tile_pool(name="ps_av", bufs=2, space="PSUM")
            )
            ps_pj = ctx.enter_context(
                tc.tile_pool(name="ps_pj", bufs=2, space="PSUM")
            )

            att_sb = ctx.enter_context(tc.tile_pool(name="att_sb", bufs=4))

            # ---------- PE work units (fillers) ----------
            def v_unit(nt_):
                def emit():
                    psv = ps_pj.tile([P, 512], f32, tag="pj", name="psv")
                    for ic in range(IC):
                        nc.tensor.matmul(
                            psv,
                            efT[:, ic, ts(nt_, P)],
                            wvT[:, ic, :],
                            start=(ic == 0),
                            stop=(ic == IC - 1),
                        )
                    nc.vector.tensor_add(
                        vaug[:, nt_, :, 0:HEAD],
                        psv[:].rearrange("p (h d) -> p h d", h=HPC),
                        bvb[:].rearrange("p (h d) -> p h d", h=HPC),
                    )

                return emit

            def chain_unit(which, oc, t2):
                wT, b_sb, actT, dstT = (
                    (wkT, bk_sb, efT, kT) if which == "k" else (wqT, bq_sb, xT, qT)
                )

                def emit():
                    psj = ps_pj.tile([P, 512], f32, tag="pj", name="psj")
                    for ic in range(IC):
                        nc.tensor.matmul(
                            psj,
                            wT[:, ic, ts(oc, P)],
                            actT[:, ic, ts(t2, 512)],
                            start=(ic == 0),
                            stop=(ic == IC - 1),
                        )
                    nc.vector.tensor_scalar_add(
                        dstT[:, oc, ts(t2, 512)], psj, b_sb[:, oc : oc + 1]
                    )

                return emit

            Es = {}

            def widths(tcv):
                out = []
                nis = range(4) if tcv == 0 else range(NT)
                for nt_ in nis:
                    t0 = nt_ * P
                    cs = max(t0, 512 * tcv)
                    w = 512 * (tcv + 1) - cs
                    out.append((nt_, cs, w))
                return out

            def av_unit(h, tcv):
                """Both s-sweeps + combine for one (head, t-chunk)."""
                def emit():
                    wlist = widths(tcv)
                    last = wlist[-1][0]
                    avp = [
                        ps_av.tile([P, 512], f32, tag="av", name="avp")
                        for _ in range(2)
                    ]
                    # interleave s-sweeps by n-tile so the final tile's MMs
                    # are the only ones gated on the last exp
                    for nt_, cs, w in wlist:
                        E = Es[(h, tcv, nt_)]
                        off = 512 - w
                        for s in range(2):
                            nc.tensor.matmul(
                                avp[s][:, off : off + w],
                                vaug[:, nt_, h, :],
                                E[:, s, :w],
                                start=(nt_ == 0),
                                stop=(nt_ == last),
                            )
                    # combine: m = P0/S0 - lambda*P1/S1, fp16 out
                    Rb = []
                    for s in range(2):
                        sc = att_sb.tile([HEAD, 512], f32, tag="sc", bufs=4, name="sc")
                        nc.vector.tensor_copy(sc, avp[s][HEAD : 2 * HEAD, :])
                        r = att_sb.tile([HEAD, 512], f32, tag="rb", bufs=4, name="rb")
                        nc.vector.reciprocal_approx_fast(out=r, in_=sc)
                        Rb.append(r)
                    m0 = att_sb.tile([HEAD, 512], f16, tag="m0", bufs=2, name="m0")
                    nc.vector.tensor_mul(m0, avp[0][0:HEAD, :], Rb[0])
                    m1 = att_sb.tile([HEAD, 512], f16, tag="m1", bufs=2, name="m1")
                    nc.vector.scalar_tensor_tensor(
                        out=m1,
                        in0=avp[1][0:HEAD, :],
                        scalar=lam_neg64,
                        in1=Rb[1],
                        op0=ALU.mult,
                        op1=ALU.mult,
                    )
                    mc = att_sb.tile([HEAD, 512], f16, tag="mc", bufs=2, name="mc")
                    nc.vector.tensor_add(mc, m0, m1)
                    nc.sync.dma_start(
                        outT_d[HEAD * h : HEAD * (h + 1), ts(tcv, 512)], mc
                    )

                return emit

            # ---------- QK + exp for one (h, tcv, nt) group ----------
            def emit_qk_group(h, tcv, nt_, cs, w):
                j = h % 2
                att_ps = ps_qk.tile([P, 2, 512], f32, tag="qk", name="attps")
                if w > 256:
                    E = att_sb.tile([P, 2, 512], f16, tag="Eb", bufs=22, name="E")
                else:
                    E = att_sb.tile([P, 2, 256], f16, tag="Es", bufs=12, name="E")
                Es[(h, tcv, nt_)] = E
                # the very first group is split so its diagonal 128 columns
                # (fed by the fast-start sub-chains) reach the scalar engine
                # ~10us before the full k0/q0 chains finish
                groups = [(cs, w)] if (h, tcv, nt_) != (0, 0, 0) else [
                    (0, P),
                    (P, 512 - P),
                ]
                for gcs, gw in groups:
                    lo = gcs - cs
                    for s in range(2):
                        base = 64 * j + 32 * s
                        nc.tensor.matmul(
                            att_ps[:, s, lo : lo + gw],
                            kT[base : base + 32, h // 2, ts(nt_, P)],
                            qT[base : base + 32, h // 2, gcs : gcs + gw],
                            start=True,
                            stop=True,
                            tile_position=(96, 0) if base == 96 else None,
                        )
                    nc.scalar.activation(
                        E[:, :, lo : lo + gw],
                        att_ps[:, :, lo : lo + gw],
                        AF.Exp,
                        bias=neg3[:, 0:1],
                        scale=SCALE,
                    )
                    if gcs == nt_ * P:
                        nc.vector.tensor_mul(E[:, :, 0:P], E[:, :, 0:P], tri2)

            # ---------- schedule ----------
            # PE warm-up: tiny dependency-free matmuls during the input-DMA
            # wait keep the HAM activity window hot so the first real chains
            # run at 2.4 GHz instead of the cold 1.2 GHz.
            for _ in range(20):
                wps = ps_pj.tile([1, 64], f32, tag="pj", name="wps")
                nc.tensor.matmul(wps, warm1[0:1, 0:1], warm1, start=True, stop=True)

            # fast-start: narrow k0/q0 sub-chains (keys n 0:128, queries
            # t 0:128) so the first QK group + exp fire as soon as the
            # weights and the first activation columns land; the remainder
            # chains complete kT/qT[oc0, t2=0] behind them.
            def sub_chain(which, c0, cw):
                wT, b_sb, actT, dstT = (
                    (wkT, bk_sb, efT, kT) if which == "k" else (wqT, bq_sb, xT, qT)
                )
                psj = ps_pj.tile([P, 512], f32, tag="pj", name="psj")
                for ic in range(IC):
                    nc.tensor.matmul(
                        psj[:, 0:cw],
                        wT[:, ic, 0:P],
                        actT[:, ic, c0 : c0 + cw],
                        start=(ic == 0),
                        stop=(ic == IC - 1),
                    )
                nc.vector.tensor_scalar_add(
                    dstT[:, 0, c0 : c0 + cw], psj[:, 0:cw], b_sb[:, 0:1]
                )

            sub_chain("k", 0, P)
            sub_chain("q", 0, P)
            sub_chain("k", P, 512 - P)
            sub_chain("q", P, 512 - P)

            K, Q, V, A = "k", "q", "v", "av"
            SLOTS = [
                # (h, tcv, [units in emission order])
                (0, 0, [(K, 0, 1), (Q, 0, 1)]),
                (0, 1, [(V, 0), (V, 1), (V, 2), (V, 3)]),
                (1, 0, [(V, 4), (A, 0, 0)]),
                (1, 1, [(K, 1, 0), (Q, 1, 0), (V, 5), (A, 1, 0)]),
                (2, 0, [(K, 1, 1), (Q, 1, 1), (V, 6)]),
                (2, 1, [(V, 7), (A, 0, 1), (A, 2, 0)]),
                (3, 0, [(K, 2, 0), (A, 1, 1)]),
                (3, 1, [(Q, 2, 0), (K, 2, 1), (A, 3, 0)]),
                (4, 0, [(Q, 2, 1), (A, 2, 1)]),
                (4, 1, [(K, 3, 0), (A, 4, 0)]),
                (5, 0, [(Q, 3, 0), (A, 3, 1)]),
                (5, 1, [(K, 3, 1), (Q, 3, 1), (A, 5, 0)]),
                (6, 0, [(A, 4, 1)]),
                (6, 1, [(A, 6, 0)]),
                (7, 0, [(A, 5, 1)]),
                (7, 1, [(A, 7, 0), (A, 6, 1)]),
            ]
            TAIL = [(A, 7, 1)]

            def make_unit(u):
                if u[0] == V:
                    return v_unit(u[1])
                if u[0] == A:
                    return av_unit(u[1], u[2])
                return chain_unit(u[0], u[1], u[2])

            for h, tcv, units in SLOTS:
                units = [make_unit(u) for u in units]
                ui = 0
                for nt_, cs, w in widths(tcv):
                    emit_qk_group(h, tcv, nt_, cs, w)
                    if ui < len(units):
                        units[ui]()
                        ui += 1
                while ui < len(units):
                    units[ui]()
                    ui += 1
            for u in TAIL:
                make_unit(u)()

    nc.compile()
    return nc


def _ensure_axon_hooks():
    """concourse's trace path imports antenv.axon_hooks, which this image
    lacks; provide it (registering the real ctypes NTFF hook when available)
    so BASS_TRACE=1 degrades gracefully instead of crashing."""
    import sys
    import types

    if "antenv.axon_hooks" in sys.modules:
        return
    try:
        import antenv.axon_hooks  # noqa: F401

        return
    except ImportError:
        pass
    mod = types.ModuleType("antenv.axon_hooks")
    mod._hook = None
    mod.set_axon_ntff_profile_hook = lambda h: setattr(mod, "_hook", h)
    mod.get_axon_ntff_profile_hook = lambda: mod._hook
    sys.modules["antenv.axon_hooks"] = mod
    import os

    if os.environ.get("KERNEL_TRACE") == "1":
        try:
            from trn_agent_boot.trn_boot import _ntff_profile_via_ctypes

            mod._hook = _ntff_profile_via_ctypes("/opt/axon/libaxon_pjrt.so")
        except Exception:
            pass


def _get_state():
    if "nc" not in _STATE:
        from concourse.bass_utils import run_bass_kernel_spmd

        _ensure_axon_hooks()
        _STATE["nc"] = _build_nc()
        _STATE["run"] = run_bass_kernel_spmd
    return _STATE


def kernel(**inputs):
    st = _get_state()

    def f32c(a):
        return np.ascontiguousarray(np.asarray(a, dtype=np.float32))

    x = np.asarray(inputs["x"], dtype=np.float32)
    ef = np.asarray(inputs["encoder_feature"], dtype=np.float32)
    Wq, bq = np.asarray(inputs["Wq"], np.float32), np.asarray(inputs["bq"], np.float32)
    Wk, bk = np.asarray(inputs["Wk"], np.float32), np.asarray(inputs["bk"], np.float32)
    Wv, bv = np.asarray(inputs["Wv"], np.float32), np.asarray(inputs["bv"], np.float32)
    lq1 = f32c(inputs["lambda_q1"]).reshape(1, HALF)
    lq2 = f32c(inputs["lambda_q2"]).reshape(1, HALF)
    lk1 = f32c(inputs["lambda_k1"]).reshape(1, HALF)
    lk2 = f32c(inputs["lambda_k2"]).reshape(1, HALF)

    in_maps = []
    for c in range(NCORES):
        b, hg = c // 2, c % 2
        sl = slice(hg * O, (hg + 1) * O)
        in_maps.append(
            {
                "xt": np.ascontiguousarray(x[b].T.astype(np.float16)),
                "eft": np.ascontiguousarray(ef[b].T.astype(np.float16)),
                "wqt": np.ascontiguousarray(Wq[sl].T.astype(np.float16)),
                "wkt": np.ascontiguousarray(Wk[sl].T.astype(np.float16)),
                "wvt": np.ascontiguousarray(Wv[sl].T.astype(np.float16)),
                "bq": f32c(bq[sl]).reshape(1, O),
                "bk": f32c(bk[sl]).reshape(1, O),
                "bv": f32c(bv[sl]).reshape(1, O),
                "lq1": lq1,
                "lq2": lq2,
                "lk1": lk1,
                "lk2": lk2,
            }
        )

    res = st["run"](st["nc"], in_maps, core_ids=list(range(NCORES)))
    _STATE["last_results"] = res

    out = np.empty((B, T, HIDDEN), dtype=np.float32)
    for c in range(NCORES):
        b, hg = c // 2, c % 2
        out[b, :, hg * O : (hg + 1) * O] = res.results[c]["outT"].T.astype(np.float32)
    return out
